# revision 1
# baseline (speedup 1.0000x reference)
"""CP-decomposition loss kernel for Trainium2 (8 NeuronCores, data parallel).

loss = sum_n (sum_r U0[i_n,r]*U1[j_n,r]*U2[k_n,r] - y_n)^2

Strategy (sharding_hint: data-parallel over the 2M observations, tables
replicated): each core processes 250k observations. Gathers use the MoE
dma_gather primitive: tables are padded to 64B rows and packed 4 rows per
256B block so block ids fit int16; the needed row is selected on the DVE
with 4 masks from s = i % 4. Rank-reduction + squared-error on DVE, final
scalar via a ones-matmul on the PE. Host sums the 8 per-core partials.
"""
import numpy as np

RANK = 10
DIM = 100000
N_OBS = 2000000
N_CORES = 8
NC_OBS = N_OBS // N_CORES          # 250000
NBLK = 2048                        # observations per gather block
NBLOCKS = -(-NC_OBS // NBLK)       # 123
NC_PAD = NBLOCKS * NBLK            # 253952
C = NBLK // 128                    # 32 free-dim slots
VB = DIM // 4 + 1                  # 25001 table blocks (last = zeros pad)
IDXF = NBLK // 16                  # 256 int16 per partition per block

_cache = {}


def _build():
    import concourse.bacc as bacc
    import concourse.bass as bass
    import concourse.mybir as mybir
    from concourse.tile import TileContext

    fp32 = mybir.dt.float32
    nc = bacc.Bacc(target_bir_lowering=False, num_swdge_queues=4)

    t4 = [nc.dram_tensor(f"t4_{t}", [VB, 64], fp32, kind="ExternalInput")
          for t in range(3)]
    bidx = [nc.dram_tensor(f"b16_{t}", [NBLOCKS, 128, IDXF], mybir.dt.int16,
                           kind="ExternalInput") for t in range(3)]
    sval = [nc.dram_tensor(f"s_{t}", [NBLOCKS, 128, C], fp32,
                           kind="ExternalInput") for t in range(3)]
    yv = nc.dram_tensor("yv", [NBLOCKS, 128, C], fp32, kind="ExternalInput")
    loss = nc.dram_tensor("loss", [1, 1], fp32, kind="ExternalOutput")

    with TileContext(nc) as tc:
        with tc.tile_pool(name="gp", bufs=8) as gp, \
             tc.tile_pool(name="ip", bufs=20) as ip, \
             tc.tile_pool(name="sp", bufs=10) as sp, \
             tc.tile_pool(name="op", bufs=6) as op, \
             tc.tile_pool(name="fix", bufs=1) as fix, \
             tc.tile_pool(name="ps", bufs=1, space="PSUM") as ps:
            acc = fix.tile([128, C], fp32)
            nc.vector.memset(acc[:], 0.0)
            # j4[p, j] = j  (constant for the 4-way subrow masks)
            j4 = fix.tile([128, 4], fp32)
            for j in range(4):
                nc.vector.memset(j4[:, j:j + 1], float(j))
            j4b = bass.AP(j4.tensor, j4[:].offset,
                          [j4[:].ap[0], [0, C], [1, 4]])
            for b in range(NBLOCKS):
                otiles = []
                for t in range(3):
                    it = ip.tile([128, IDXF], mybir.dt.int16, tag="idx")
                    nc.sync.dma_start(out=it[:], in_=bidx[t][b])
                    g = gp.tile([128, C * 64], fp32, tag=f"g{t}")
                    nc.gpsimd.dma_gather(
                        out_ap=g[:].rearrange("p (c e) -> p c e", e=64),
                        in_ap=t4[t][:],
                        idxs_ap=it[:],
                        num_idxs=NBLK,
                        num_idxs_reg=NBLK,
                        elem_size=64,
                        single_packet=False,
                        queue_num=(3 * b + t) % 4,
                    )
                    st = sp.tile([128, C], fp32, tag="s")
                    nc.sync.dma_start(out=st[:], in_=sval[t][b])
                    # masks M[p, c, j] = (s[p, c] == j)
                    m4 = op.tile([128, C * 4], fp32, tag="m4")
                    stb = bass.AP(st.tensor, st[:].offset,
                                  [st[:].ap[0], [1, C], [0, 4]])
                    nc.vector.tensor_tensor(
                        out=m4[:].rearrange("p (c j) -> p c j", j=4),
                        in0=stb, in1=j4b, op=mybir.AluOpType.is_equal)
                    # masked product over the 4 subrow candidates, j innermost
                    gj = bass.AP(g.tensor, g[:].offset,
                                 [g[:].ap[0], [64, C], [1, RANK], [16, 4]])
                    mj = bass.AP(m4.tensor, m4[:].offset,
                                 [m4[:].ap[0], [4, C], [0, RANK], [1, 4]])
                    tmp = op.tile([128, C * RANK * 4], fp32, tag="tmp")
                    nc.vector.tensor_tensor(
                        out=tmp[:].rearrange("p (c r j) -> p c r j", r=RANK, j=4),
                        in0=gj, in1=mj, op=mybir.AluOpType.mult)
                    ot = op.tile([128, C * RANK], fp32, tag=f"o{t}")
                    nc.vector.tensor_reduce(
                        out=ot[:],
                        in_=tmp[:].rearrange("p (c r j) -> p c r j", r=RANK, j=4),
                        axis=mybir.AxisListType.X, op=mybir.AluOpType.add)
                    otiles.append(ot)
                prod = op.tile([128, C * RANK], fp32, tag="prod")
                nc.vector.tensor_mul(out=prod[:], in0=otiles[0][:], in1=otiles[1][:])
                nc.vector.tensor_mul(out=prod[:], in0=prod[:], in1=otiles[2][:])
                pred = op.tile([128, C], fp32, tag="pred")
                nc.vector.tensor_reduce(
                    out=pred[:],
                    in_=prod[:].rearrange("p (c r) -> p c r", r=RANK),
                    axis=mybir.AxisListType.X, op=mybir.AluOpType.add)
                yt = sp.tile([128, C], fp32, tag="y")
                nc.sync.dma_start(out=yt[:], in_=yv[b])
                d = op.tile([128, C], fp32, tag="d")
                nc.vector.tensor_tensor(out=d[:], in0=pred[:], in1=yt[:],
                                        op=mybir.AluOpType.subtract)
                nc.vector.tensor_mul(out=d[:], in0=d[:], in1=d[:])
                nc.vector.tensor_add(out=acc[:], in0=acc[:], in1=d[:])
            accr = fix.tile([128, 1], fp32)
            nc.vector.tensor_reduce(out=accr[:], in_=acc[:],
                                    axis=mybir.AxisListType.X,
                                    op=mybir.AluOpType.add)
            ones = fix.tile([128, 1], fp32)
            nc.vector.memset(ones[:], 1.0)
            pt = ps.tile([1, 1], fp32, space="PSUM")
            nc.tensor.matmul(out=pt[:], lhsT=accr[:], rhs=ones[:],
                             start=True, stop=True)
            res = fix.tile([1, 1], fp32)
            nc.vector.tensor_copy(out=res[:], in_=pt[:])
            nc.sync.dma_start(out=loss[:], in_=res[:])
    nc.compile()
    return nc


def _prep_table(u):
    t4 = np.zeros((VB, 64), dtype=np.float32)
    v = t4[:DIM // 4].reshape(DIM // 4, 4, 16)
    v[:, :, :RANK] = np.asarray(u, dtype=np.float32).reshape(DIM // 4, 4, RANK)
    return t4


def kernel(indices, y, U0, U1, U2):
    from concourse.bass_utils import run_bass_kernel_spmd

    if "nc" not in _cache:
        _cache["nc"] = _build()
    nc = _cache["nc"]

    indices = np.asarray(indices)
    y = np.asarray(y, dtype=np.float32)
    t4s = [_prep_table(u) for u in (U0, U1, U2)]

    in_maps = []
    for c in range(N_CORES):
        sl = slice(c * NC_OBS, (c + 1) * NC_OBS)
        m = {f"t4_{t}": t4s[t] for t in range(3)}
        for t in range(3):
            it = np.asarray(indices[sl, t], dtype=np.int64)
            b = (it >> 2).astype(np.int16)
            s = (it & 3).astype(np.float32)
            b_pad = np.full(NC_PAD, DIM // 4, dtype=np.int16)
            b_pad[:NC_OBS] = b
            s_pad = np.zeros(NC_PAD, dtype=np.float32)
            s_pad[:NC_OBS] = s
            # wrapped int16 layout: position m -> (partition m%16, free m//16),
            # replicated across the 8 16-partition groups
            w = b_pad.reshape(NBLOCKS, IDXF, 16).transpose(0, 2, 1)
            m[f"b16_{t}"] = np.tile(w, (1, 8, 1)).copy()
            # slot layout: obs m of a block sits at (partition m%128, free m//128)
            m[f"s_{t}"] = s_pad.reshape(NBLOCKS, C, 128).transpose(0, 2, 1).copy()
        y_pad = np.zeros(NC_PAD, dtype=np.float32)
        y_pad[:NC_OBS] = y[sl]
        m["yv"] = y_pad.reshape(NBLOCKS, C, 128).transpose(0, 2, 1).copy()
        in_maps.append(m)

    global _last_in_maps
    _last_in_maps = in_maps
    res = run_bass_kernel_spmd(nc, in_maps, core_ids=list(range(N_CORES)))
    total = np.float32(0.0)
    for c in range(N_CORES):
        total += res.results[c]["loss"][0, 0]
    return np.float32(total)



# revision 2
# speedup vs baseline: 1.0994x; 1.0994x over previous
"""CP-decomposition loss kernel v6: windowed-U0 hybrid.

Observations are sorted by the table-0 index. Groups of up to 8
consecutive sorted observations share one 512B window (8 table-0 rows,
256B-aligned base) fetched with a single dma_gather descriptor via
elem_step=64 < elem_size=128. Tables 1/2 keep the per-observation 256B
block gather + 4-way subrow select. The loss is order-invariant, so the
sort needs no unsort. ~8x fewer table-0 descriptors on the Q7 SWDGE,
which the baseline trace showed to be the serial bottleneck.
"""
import numpy as np

RANK = 10
DIM = 100000
N_OBS = 2000000
N_CORES = 8
NC_OBS = N_OBS // N_CORES          # 250000
NBLK = 2048                        # observation slots per block
C = NBLK // 128                    # 32 slot columns per partition
GW = C // 8                        # 4 window groups per partition
NG = NBLK // 8                     # 512 window groups per block
VB = DIM // 4 + 2                  # 25002 table blocks (last two zeros)
ZBLK = DIM // 4                    # first zero block id (25000)
IDXF = NBLK // 16                  # int16 idx per partition (U1/U2)
IDXF0 = NG // 16                   # int16 idx per partition (U0 windows)

_cache = {}


def _build(nblocks):
    import concourse.bacc as bacc
    import concourse.bass as bass
    import concourse.mybir as mybir
    from concourse.tile import TileContext

    fp32 = mybir.dt.float32
    nc = bacc.Bacc(target_bir_lowering=False, num_swdge_queues=4)

    t4 = [nc.dram_tensor(f"t4_{t}", [VB, 64], fp32, kind="ExternalInput")
          for t in range(3)]
    widx = nc.dram_tensor("widx", [nblocks, 128, IDXF0], mybir.dt.int16,
                          kind="ExternalInput")
    bidx = [nc.dram_tensor(f"b16_{t}", [nblocks, 128, IDXF], mybir.dt.int16,
                           kind="ExternalInput") for t in (1, 2)]
    sval = [nc.dram_tensor(f"s_{t}", [nblocks, 128, C], fp32,
                           kind="ExternalInput") for t in range(3)]
    yv = nc.dram_tensor("yv", [nblocks, 128, C], fp32, kind="ExternalInput")
    loss = nc.dram_tensor("loss", [1, 1], fp32, kind="ExternalOutput")

    with TileContext(nc) as tc:
        with tc.tile_pool(name="gp", bufs=4) as gp, \
             tc.tile_pool(name="ip", bufs=10) as ip, \
             tc.tile_pool(name="sp", bufs=6) as sp, \
             tc.tile_pool(name="op", bufs=4) as op, \
             tc.tile_pool(name="fix", bufs=1) as fix, \
             tc.tile_pool(name="ps", bufs=1, space="PSUM") as ps:
            acc = fix.tile([128, C], fp32)
            nc.vector.memset(acc[:], 0.0)
            # j4[p, j] = j for the 4-way masks; j8[p, s] = s for the 8-way
            j4 = fix.tile([128, 4], fp32)
            for j in range(4):
                nc.vector.memset(j4[:, j:j + 1], float(j))
            j4b = bass.AP(j4.tensor, j4[:].offset,
                          [j4[:].ap[0], [0, C], [1, 4]])
            j8 = fix.tile([128, 8], fp32)
            for j in range(8):
                nc.vector.memset(j8[:, j:j + 1], float(j))
            j8b = bass.AP(j8.tensor, j8[:].offset,
                          [j8[:].ap[0], [0, C], [1, 8]])
            for b in range(nblocks):
                otiles = []
                # --- table 0: windowed gather (512 descs for 4096 slots)
                it0 = ip.tile([128, IDXF0], mybir.dt.int16, tag="idx0")
                nc.sync.dma_start(out=it0[:], in_=widx[b])
                g0 = gp.tile([128, GW * 128], fp32, tag="g0")
                in0 = bass.AP(t4[0], 0, [[64, VB * 64 // 64 - 1], [1, 128]])
                nc.gpsimd.dma_gather(
                    out_ap=g0[:].rearrange("p (c e) -> p c e", e=128),
                    in_ap=in0,
                    idxs_ap=it0[:],
                    num_idxs=NG,
                    num_idxs_reg=NG,
                    elem_size=128,
                    elem_step=64,
                    single_packet=False,
                    queue_num=(3 * b) % 4,
                )
                st0 = sp.tile([128, C], fp32, tag="s0")
                nc.sync.dma_start(out=st0[:], in_=sval[0][b])
                # masks M0[p, c, s] = (s0[p, c] == s), s in [0, 8)
                m0 = op.tile([128, C * 8], fp32, tag="m0")
                stb0 = bass.AP(st0.tensor, st0[:].offset,
                               [st0[:].ap[0], [1, C], [0, 8]])
                nc.vector.tensor_tensor(
                    out=m0[:].rearrange("p (c s) -> p c s", s=8),
                    in0=stb0, in1=j8b, op=mybir.AluOpType.is_equal)
                ot0 = op.tile([128, C * RANK], fp32, tag="o0")
                for w in range(GW):
                    gj = bass.AP(g0.tensor, g0[:].offset + w * 128,
                                 [g0[:].ap[0], [0, 8], [1, RANK], [16, 8]])
                    mj = bass.AP(m0.tensor, m0[:].offset + w * 64,
                                 [m0[:].ap[0], [8, 8], [0, RANK], [1, 8]])
                    tmp = op.tile([128, 8 * RANK * 8], fp32, tag="tmp0")
                    nc.vector.tensor_tensor(
                        out=tmp[:].rearrange("p (j r s) -> p j r s",
                                             r=RANK, s=8),
                        in0=gj, in1=mj, op=mybir.AluOpType.mult)
                    nc.vector.tensor_reduce(
                        out=ot0[:, w * 8 * RANK:(w + 1) * 8 * RANK],
                        in_=tmp[:].rearrange("p (j r s) -> p j r s",
                                             r=RANK, s=8),
                        axis=mybir.AxisListType.X, op=mybir.AluOpType.add)
                otiles.append(ot0)
                # --- tables 1, 2: per-slot 256B block gather + 4-way select
                for t in (1, 2):
                    it = ip.tile([128, IDXF], mybir.dt.int16, tag="idx")
                    nc.sync.dma_start(out=it[:], in_=bidx[t - 1][b])
                    g = gp.tile([128, C * 64], fp32, tag=f"g{t}")
                    nc.gpsimd.dma_gather(
                        out_ap=g[:].rearrange("p (c e) -> p c e", e=64),
                        in_ap=t4[t][:],
                        idxs_ap=it[:],
                        num_idxs=NBLK,
                        num_idxs_reg=NBLK,
                        elem_size=64,
                        single_packet=False,
                        queue_num=(3 * b + t) % 4,
                    )
                    st = sp.tile([128, C], fp32, tag="s")
                    nc.sync.dma_start(out=st[:], in_=sval[t][b])
                    m4 = op.tile([128, C * 4], fp32, tag="m4")
                    stb = bass.AP(st.tensor, st[:].offset,
                                  [st[:].ap[0], [1, C], [0, 4]])
                    nc.vector.tensor_tensor(
                        out=m4[:].rearrange("p (c j) -> p c j", j=4),
                        in0=stb, in1=j4b, op=mybir.AluOpType.is_equal)
                    gj = bass.AP(g.tensor, g[:].offset,
                                 [g[:].ap[0], [64, C], [1, RANK], [16, 4]])
                    mj = bass.AP(m4.tensor, m4[:].offset,
                                 [m4[:].ap[0], [4, C], [0, RANK], [1, 4]])
                    tmp = op.tile([128, C * RANK * 4], fp32, tag="tmp")
                    nc.vector.tensor_tensor(
                        out=tmp[:].rearrange("p (c r j) -> p c r j",
                                             r=RANK, j=4),
                        in0=gj, in1=mj, op=mybir.AluOpType.mult)
                    ot = op.tile([128, C * RANK], fp32, tag=f"o{t}")
                    nc.vector.tensor_reduce(
                        out=ot[:],
                        in_=tmp[:].rearrange("p (c r j) -> p c r j",
                                             r=RANK, j=4),
                        axis=mybir.AxisListType.X, op=mybir.AluOpType.add)
                    otiles.append(ot)
                prod = op.tile([128, C * RANK], fp32, tag="prod")
                nc.vector.tensor_mul(out=prod[:], in0=otiles[0][:],
                                     in1=otiles[1][:])
                nc.vector.tensor_mul(out=prod[:], in0=prod[:],
                                     in1=otiles[2][:])
                pred = op.tile([128, C], fp32, tag="pred")
                nc.vector.tensor_reduce(
                    out=pred[:],
                    in_=prod[:].rearrange("p (c r) -> p c r", r=RANK),
                    axis=mybir.AxisListType.X, op=mybir.AluOpType.add)
                yt = sp.tile([128, C], fp32, tag="y")
                nc.sync.dma_start(out=yt[:], in_=yv[b])
                d = op.tile([128, C], fp32, tag="d")
                nc.vector.tensor_tensor(out=d[:], in0=pred[:], in1=yt[:],
                                        op=mybir.AluOpType.subtract)
                nc.vector.tensor_mul(out=d[:], in0=d[:], in1=d[:])
                nc.vector.tensor_add(out=acc[:], in0=acc[:], in1=d[:])
            accr = fix.tile([128, 1], fp32)
            nc.vector.tensor_reduce(out=accr[:], in_=acc[:],
                                    axis=mybir.AxisListType.X,
                                    op=mybir.AluOpType.add)
            ones = fix.tile([128, 1], fp32)
            nc.vector.memset(ones[:], 1.0)
            pt = ps.tile([1, 1], fp32, space="PSUM")
            nc.tensor.matmul(out=pt[:], lhsT=accr[:], rhs=ones[:],
                             start=True, stop=True)
            res = fix.tile([1, 1], fp32)
            nc.vector.tensor_copy(out=res[:], in_=pt[:])
            nc.sync.dma_start(out=loss[:], in_=res[:])
    nc.compile()
    return nc


def _prep_table(u):
    t4 = np.zeros((VB, 64), dtype=np.float32)
    v = t4[:DIM // 4].reshape(DIM // 4, 4, 16)
    v[:, :, :RANK] = np.asarray(u, dtype=np.float32).reshape(DIM // 4, 4, RANK)
    return t4


def _group_core(i_sorted):
    """Greedy-pack sorted table-0 rows into <=8-obs groups sharing a
    256B-aligned 8-row window. Returns (win_base_block, n_groups,
    group_of_obs, subrow_of_obs, slot_j_of_obs) as flat arrays."""
    n = len(i_sorted)
    wb = []
    grp = np.empty(n, dtype=np.int64)
    sub = np.empty(n, dtype=np.int64)
    slj = np.empty(n, dtype=np.int64)
    pos = 0
    while pos < n:
        base = (i_sorted[pos] >> 2)          # 4-row-aligned block id
        hi = base * 4 + 8                    # window covers [4b, 4b+8)
        g = len(wb)
        cnt = 0
        while pos < n and cnt < 8 and i_sorted[pos] < hi:
            grp[pos] = g
            sub[pos] = i_sorted[pos] - base * 4
            slj[pos] = cnt
            cnt += 1
            pos += 1
        wb.append(base)
    return np.asarray(wb, dtype=np.int64), grp, sub, slj


def kernel(indices, y, U0, U1, U2):
    from concourse.bass_utils import run_bass_kernel_spmd

    indices = np.asarray(indices)
    y = np.asarray(y, dtype=np.float32)
    t4s = [_prep_table(u) for u in (U0, U1, U2)]

    in_maps = []
    nblocks_all = None
    per_core = []
    for c in range(N_CORES):
        sl = slice(c * NC_OBS, (c + 1) * NC_OBS)
        idx_c = np.asarray(indices[sl], dtype=np.int64)
        y_c = y[sl]
        order = np.argsort(idx_c[:, 0], kind="stable")
        i_sorted = idx_c[order, 0]
        wb, grp, sub, slj = _group_core(i_sorted)
        ngrp = len(wb)
        nblocks = -(-ngrp // NG)
        per_core.append((idx_c, y_c, order, wb, grp, sub, slj, nblocks))
        nblocks_all = max(nblocks_all or 0, nblocks)

    key = ("nc", nblocks_all)
    if _cache.get("key") != key:
        _cache["nc"] = _build(nblocks_all)
        _cache["key"] = key
    nc = _cache["nc"]

    for c in range(N_CORES):
        idx_c, y_c, order, wb, grp, sub, slj, nblocks = per_core[c]
        nblocks = nblocks_all
        ngrp_pad = nblocks * NG
        # window id list, padded with the zero block
        wb_pad = np.full(ngrp_pad, ZBLK, dtype=np.int16)
        wb_pad[:len(wb)] = wb.astype(np.int16)
        # slot position of each (sorted) obs: group g -> block b, local gb;
        # partition gb%128, col (gb//128)*8 + j
        g_all = grp
        b_of = g_all // NG
        gb = g_all % NG
        part = gb % 128
        col = (gb // 128) * 8 + slj
        # per-slot arrays [nblocks, 128, C]
        s0 = np.zeros((nblocks, 128, C), dtype=np.float32)
        bi = [np.full((nblocks, 128, C), ZBLK, dtype=np.int64) for _ in range(2)]
        sv = [np.zeros((nblocks, 128, C), dtype=np.float32) for _ in range(2)]
        yv_ = np.zeros((nblocks, 128, C), dtype=np.float32)
        s0[b_of, part, col] = sub.astype(np.float32)
        for t in (1, 2):
            it = idx_c[order, t]
            bi[t - 1][b_of, part, col] = it >> 2
            sv[t - 1][b_of, part, col] = (it & 3).astype(np.float32)
        yv_[b_of, part, col] = y_c[order]
        # pads: s0 stays 0 -> selects window subrow 0 (real values), but
        # tables 1/2 select the zero block -> product = 0, y = 0.
        m = {f"t4_{t}": t4s[t] for t in range(3)}
        w16 = wb_pad.reshape(nblocks, IDXF0, 16).transpose(0, 2, 1)
        m["widx"] = np.tile(w16, (1, 8, 1)).copy()
        for t in (1, 2):
            bt = bi[t - 1].transpose(0, 1, 2)  # already [nb, 128, C]
            # gather idx list: position m -> slot (m%128, m//128):
            # list value at position m = block id of that slot
            flat = np.empty((nblocks, NBLK), dtype=np.int16)
            flat[:, :] = bt.transpose(0, 2, 1).reshape(nblocks, NBLK)
            w = flat.reshape(nblocks, IDXF, 16).transpose(0, 2, 1)
            m[f"b16_{t}"] = np.tile(w, (1, 8, 1)).copy()
            m[f"s_{t}"] = sv[t - 1]
        m["s_0"] = s0
        m["yv"] = yv_
        in_maps.append(m)

    global _last_in_maps
    _last_in_maps = in_maps
    res = run_bass_kernel_spmd(nc, in_maps, core_ids=list(range(N_CORES)))
    total = np.float32(0.0)
    for c in range(N_CORES):
        total += res.results[c]["loss"][0, 0]
    return np.float32(total)
